# revision 10
# baseline (speedup 1.0000x reference)
"""Trainium2 Bass kernel for NpuQuantizationLinear.

Reference semantics (bit-exact target):
    qx  = clip(round_half_even(x * act_scale + act_offset), -128, 127)  # int8
    acc = qx @ q_weight  (int8 x int8 -> int32 accumulation)
    out = (acc + bias_i32) * deq_scale                                   # f32

Implementation notes:
  * Sharding: rows of x (M) are split across the 8 cores.  Column-parallel
    (the hint) would replicate the 128 MB x load + the quantize work on
    every core; row-parallel loads x once total and keeps all per-core
    work 1/8th.  No collective needed either way.
  * The PE has no int8 mode, but every int8 value is exactly representable
    in bf16 and the PSUM accumulates in fp32, which is exact for integer
    partial sums below 2^24 (|acc| here is ~1e5-1e6).  So a bf16 matmul of
    the quantized operands reproduces the int32 GEMM bit-exactly.
  * Quantize is split across two engines: the Act engine computes
    t = Identity(x*s + 0) (the affine is a single-rounding FMA; bias 0
    keeps it equal to a plain f32 multiply), the DVE does the fused
    round-half-even (t + 1.5*2^23 - 1.5*2^23) in one two-ALU pass.
  * x is staged chunk-major ([KT/4, 128, 4*MP]) so each x DMA moves a
    contiguous 2 MiB (small transfers only reach ~270 GB/s; >=2 MiB gets
    ~330+).  The first chunk is split so kt0 lands fast and the PE can
    start.  All weight loads ride the scalar-engine HWDGE ring so x
    streams uncontended on the sync ring.
  * Warm-up: 4 n-tiles accumulate (all 8 PSUM banks) with the k-loop
    outer, joining as their weight tile arrives (missed k-tiles are
    replayed at the end of phase 1 -- fp32 accumulation of <2^24-magnitude
    integers is exact in any order).  Steady state runs kt-outer/mb-inner
    so consecutive matmuls share the stationary tile.
  * Output is computed transposed ([N, M] per core) so bias/deq are
    per-partition scalars -> single fused tensor_scalar epilogue
    (acc + bias) * deq, one rounding, matching the reference exactly.
  * Host-side work is layout-only: transpose/slice x, int8->bf16 cast and
    swizzle of the weight, un-transpose of the output.
"""

import numpy as np
import ml_dtypes

_NC = 8  # NeuronCores
_P = 128  # partitions
_FREE = 512  # matmul moving free dim / PSUM bank (fp32)
_CH = 4  # k-tiles per x DMA chunk
_MAGIC = 12582912.0  # 1.5 * 2**23, RNE round-to-int magic constant

_nc_cache = {}


def _build_bass(
    MP, KT, NT, act_scale, act_offset, need_clip, nt_warm, body_reps=1, loop_trips=1,
    internal_io=False,
):
    """Emit the per-core Bass/Tile program.

    DRAM tensors (per core):
      xt     [KT/CH, 128, CH*MP] f32  x-slice transposed, chunk-major
      w      [NT, 128, KT, 128] bf16  weight swizzled per n-tile
      bias_s [128, NT] f32   bias striped: [p, nt] = bias[nt*128 + p]
      deq_s  [128, NT] f32   deq striped likewise
      out    [NT, 128, MP] f32  transposed output: [nt, p, m] = y[m, nt*128+p]
    """
    from contextlib import ExitStack

    import concourse.mybir as mybir
    import concourse.tile as tile
    from concourse import bacc

    f32 = mybir.dt.float32
    bf16 = mybir.dt.bfloat16
    Alu = mybir.AluOpType
    Identity = mybir.ActivationFunctionType.Identity
    MB = MP // _FREE
    NCH = KT // _CH

    nc = bacc.Bacc("TRN2", target_bir_lowering=False, debug=False)

    # internal_io: timing-only mode.  The big tensors become DRAM scratch so
    # the benchmark harness stages ~nothing over the axon tunnel; a tiny
    # dummy in/out pair keeps the PJRT wrapper well-formed.  The emitted
    # compute/DMA stream is identical to the graded build.
    kin = "Internal" if internal_io else "ExternalInput"
    kout = "Internal" if internal_io else "ExternalOutput"
    xt_d = nc.dram_tensor("xt", [NCH, _P, _CH * MP], f32, kind=kin).ap()
    w_d = nc.dram_tensor("w", [NT, _P, KT, _P], bf16, kind=kin).ap()
    bias_d = nc.dram_tensor("bias_s", [_P, NT], f32, kind=kin).ap()
    deq_d = nc.dram_tensor("deq_s", [_P, NT], f32, kind=kin).ap()
    out_d = nc.dram_tensor("out", [NT, _P, MP], f32, kind=kout).ap()
    if internal_io:
        din = nc.dram_tensor("dummy_in", [1, 4], f32, kind="ExternalInput").ap()
        dout = nc.dram_tensor("dummy_out", [1, 4], f32, kind="ExternalOutput").ap()

    with tile.TileContext(nc) as tc, ExitStack() as ctx:
        const_pool = ctx.enter_context(tc.tile_pool(name="const", bufs=1))
        qx_pool = ctx.enter_context(tc.tile_pool(name="qxp", bufs=1))
        x_pool = ctx.enter_context(tc.tile_pool(name="xp", bufs=3))
        t_pool = ctx.enter_context(tc.tile_pool(name="tp", bufs=3))
        w_pool = ctx.enter_context(tc.tile_pool(name="wp", bufs=6))
        o_pool = ctx.enter_context(tc.tile_pool(name="op", bufs=6))
        ps_pool = ctx.enter_context(tc.tile_pool(name="pp", bufs=8, space="PSUM"))

        bias_t = const_pool.tile([_P, NT], f32, name="bias_t")
        nc.sync.dma_start(bias_t[:], bias_d)
        deq_t = const_pool.tile([_P, NT], f32, name="deq_t")
        nc.sync.dma_start(deq_t[:], deq_d)
        if internal_io:
            dummy_t = const_pool.tile([1, 4], f32, name="dummy_t")
            nc.sync.dma_start(dummy_t[:], din)
            nc.sync.dma_start(dout, dummy_t[:])

        # quantized-transposed activations, resident: [128, KT, MP] bf16
        qx = qx_pool.tile([_P, KT, MP], bf16, name="qx")

        pools = dict(
            x=x_pool, t=t_pool, w=w_pool, o=o_pool, ps=ps_pool,
        )

        def emit_reps():
            for _rep in range(body_reps):
                _emit_body(
                    nc, mybir, KT, NT, MP, MB, NCH, nt_warm,
                    act_scale, act_offset, need_clip,
                    qx, bias_t, deq_t,
                    xt_d, w_d, out_d, pools,
                )

        if loop_trips > 1:
            with tc.For_i(0, loop_trips, 1):
                emit_reps()
        else:
            emit_reps()

    nc.compile()
    return nc


def _emit_body(
    nc, mybir, KT, NT, MP, MB, NCH, nt_warm,
    act_scale, act_offset, need_clip,
    qx, bias_t, deq_t,
    xt_d, w_d, out_d, pools,
):
    f32 = mybir.dt.float32
    bf16 = mybir.dt.bfloat16
    Alu = mybir.AluOpType
    Identity = mybir.ActivationFunctionType.Identity

    def load_w(nt, split=False):
        # scalar-ring HWDGE: weight traffic never queues behind x chunks
        wt = pools["w"].tile([_P, KT, _P], bf16, name="wt")
        if split and KT > 4:
            # first k-block lands fast so the PE can start early
            nc.scalar.dma_start(wt[:, 0:4, :], w_d[nt][:, 0:4, :])
            nc.scalar.dma_start(wt[:, 4:, :], w_d[nt][:, 4:, :])
        else:
            nc.scalar.dma_start(wt[:], w_d[nt])
        return wt

    def quantize(xt_c, j, kt):
        # Act engine: t = Identity(x*s + 0) -- single-rounding FMA with
        # bias 0 == plain f32 multiply.  DVE: fused +MAGIC,-MAGIC RNE.
        t1 = pools["t"].tile([_P, MP], f32, name="t1")
        t2 = pools["t"].tile([_P, MP], f32, name="t2") if need_clip else None
        for h in range(2):
            hs = slice(h * (MP // 2), (h + 1) * (MP // 2))
            src = slice(j * MP + h * (MP // 2), j * MP + (h + 1) * (MP // 2))
            nc.scalar.activation(t1[:, hs], xt_c[:, src], Identity, 0.0, act_scale)
            if act_offset != 0.0:
                # reference computes mul and add with separate roundings
                nc.vector.tensor_scalar_add(t1[:, hs], t1[:, hs], act_offset)
            if need_clip:
                nc.vector.tensor_scalar(
                    t2[:, hs], t1[:, hs], _MAGIC, _MAGIC, Alu.add, Alu.subtract
                )
                nc.vector.tensor_scalar(
                    qx[:, kt, hs], t2[:, hs], 127.0, -128.0, Alu.min, Alu.max
                )
            else:
                nc.vector.tensor_scalar(
                    qx[:, kt, hs], t1[:, hs], _MAGIC, _MAGIC, Alu.add, Alu.subtract
                )

    def epilogue(nt, mb, ps):
        ot = pools["o"].tile([_P, _FREE], f32, name="ot")
        # (acc + bias) * deq, per-partition scalars, single instruction
        nc.vector.tensor_scalar(
            ot[:], ps[:], bias_t[:, nt : nt + 1], deq_t[:, nt : nt + 1],
            Alu.add, Alu.mult,
        )
        nc.sync.dma_start(out_d[nt, :, mb * _FREE : (mb + 1) * _FREE], ot[:])

    joins = tuple(min(3 * i, KT - 1) for i in range(nt_warm))

    warm_w = [load_w(nt, split=(nt == 0)) for nt in range(nt_warm)]
    warm_ps = [
        [pools["ps"].tile([_P, _FREE], f32, name="ps") for _ in range(MB)]
        for _ in range(nt_warm)
    ]

    # PE prewarm: tiny fp32 dummy matmuls on the resident bias tile keep
    # the HAM clock gate open through the DMA-bound startup window, so the
    # real matmuls start at 2.4 GHz instead of ramping from 1.2.  Warm
    # group 0's start=True scrubs the scratch bank; zero extra DMAs.
    pw_cols = min(32, NT)
    for _ in range(28):
        nc.tensor.matmul(
            warm_ps[0][0][0:pw_cols, 0:pw_cols],
            bias_t[:, 0:pw_cols],
            bias_t[:, 0:pw_cols],
            start=True, stop=True,
        )

    def warm_mm(nt, k, start, stop):
        for mb in range(MB):
            nc.tensor.matmul(
                warm_ps[nt][mb][:], warm_w[nt][:, k, :],
                qx[:, k, mb * _FREE : (mb + 1) * _FREE],
                start=start, stop=stop,
            )

    kt = 0
    for c in range(NCH):
        xt_c = pools["x"].tile([_P, _CH * MP], f32, name="xt_c")
        if c == 0:
            # split so kt0 lands fast and the PE can start early
            nc.sync.dma_start(xt_c[:, 0:MP], xt_d[0][:, 0:MP])
            nc.sync.dma_start(xt_c[:, MP:], xt_d[0][:, MP:])
        else:
            nc.sync.dma_start(xt_c[:], xt_d[c])
        for j in range(_CH):
            quantize(xt_c, j, kt)
            for nt in range(nt_warm):
                fk = joins[nt]
                if kt < fk:
                    continue
                if kt == fk and fk > 0:
                    # join: current kt starts the group, then the whole
                    # backlog is emitted as ready filler work for the PE
                    # (fp32 accumulation of <2^24-magnitude integers is
                    # exact in any order).  If the join IS the last k-tile,
                    # the final backlog matmul closes the group.
                    warm_mm(nt, kt, start=True, stop=False)
                    for k in range(fk):
                        warm_mm(
                            nt, k, start=False,
                            stop=(fk == KT - 1 and k == fk - 1),
                        )
                else:
                    warm_mm(
                        nt, kt,
                        start=(kt == 0 and fk == 0),
                        stop=(kt == KT - 1),
                    )
            kt += 1
    for nt in range(nt_warm):
        for mb in range(MB):
            epilogue(nt, mb, warm_ps[nt][mb])

    # Steady state: kt outer / mb inner so consecutive matmuls share the
    # stationary W tile.  One PSUM bank per (nt, mb) accumulation group.
    # The last group runs mb outer so its first epilogue and store overlap
    # the remaining matmuls (drain trim).
    for nt in range(nt_warm, NT):
        wt = load_w(nt)
        pss = [pools["ps"].tile([_P, _FREE], f32, name="ps") for _ in range(MB)]
        if nt == NT - 1:
            for mb in range(MB):
                for k in range(KT):
                    nc.tensor.matmul(
                        pss[mb][:], wt[:, k, :],
                        qx[:, k, mb * _FREE : (mb + 1) * _FREE],
                        start=(k == 0), stop=(k == KT - 1),
                    )
                epilogue(nt, mb, pss[mb])
        else:
            for k in range(KT):
                for mb in range(MB):
                    nc.tensor.matmul(
                        pss[mb][:], wt[:, k, :],
                        qx[:, k, mb * _FREE : (mb + 1) * _FREE],
                        start=(k == 0), stop=(k == KT - 1),
                    )
            for mb in range(MB):
                epilogue(nt, mb, pss[mb])


def _stage_x(x, c, MP, KT):
    """Per-core chunk-major x: [KT/CH, 128, CH*MP]."""
    xs = x[c * MP : (c + 1) * MP, :].T.reshape(KT, _P, MP)
    return np.ascontiguousarray(
        xs.reshape(KT // _CH, _CH, _P, MP).transpose(0, 2, 1, 3).reshape(
            KT // _CH, _P, _CH * MP
        )
    )


def run(inputs, trace=False):
    """Full-input entry: shard, run on 8 cores, gather.  Returns (out, results)."""
    from concourse import bass_utils

    x = np.ascontiguousarray(np.asarray(inputs["x"], dtype=np.float32))
    qw = np.asarray(inputs["q_weight"])
    act_scale = float(np.asarray(inputs["act_scale"]))
    act_offset = float(np.asarray(inputs["act_offset"]))
    deq = np.asarray(inputs["deq_scale"], dtype=np.float32)
    bias = np.asarray(inputs["bias_i32"])

    M, K = x.shape
    K2, N = qw.shape
    assert K == K2 and M % _NC == 0
    MP = M // _NC
    assert MP % _FREE == 0 and K % (_P * _CH) == 0 and N % _P == 0
    KT, NT = K // _P, N // _P
    nt_warm = min(4, NT)

    # clip is a no-op unless |x*s + o| can reach 127.5; check the actual data
    t_max = float(np.abs(x).max()) * abs(act_scale) + abs(act_offset)
    need_clip = t_max >= 127.0
    assert np.abs(bias).max() < 2**24  # int32 bias must be f32-exact

    key = (MP, KT, NT, act_scale, act_offset, need_clip, nt_warm)
    if key not in _nc_cache:
        _nc_cache[key] = _build_bass(*key)
    nc = _nc_cache[key]

    # ---- host staging (layout-only transforms) ----
    # weight: int8 -> bf16 (exact), swizzled to [nt, p, kt, n]
    w_bf = qw.astype(ml_dtypes.bfloat16)
    w_r = np.ascontiguousarray(
        w_bf.reshape(KT, _P, NT, _P).transpose(2, 1, 0, 3)
    )
    bias_r = np.ascontiguousarray(bias.astype(np.float32).reshape(NT, _P).T)
    deq_r = np.ascontiguousarray(deq.reshape(NT, _P).T)

    in_maps = []
    for c in range(_NC):
        in_maps.append(
            {
                "xt": _stage_x(x, c, MP, KT),
                "w": w_r,
                "bias_s": bias_r,
                "deq_s": deq_r,
            }
        )

    results = bass_utils.run_bass_kernel_spmd(
        nc, in_maps, core_ids=list(range(_NC)), trace=trace
    )

    out = np.empty((M, N), dtype=np.float32)
    for c in range(_NC):
        out[c * MP : (c + 1) * MP, :] = results.results[c]["out"].reshape(N, MP).T
    return out, results


def kernel(**inputs) -> np.ndarray:
    out, _ = run(inputs, trace=False)
    return out


# revision 16
# speedup vs baseline: 1.2016x; 1.2016x over previous
"""Trainium2 Bass kernel for NpuQuantizationLinear.

Reference semantics (bit-exact target):
    qx  = clip(round_half_even(x * act_scale + act_offset), -128, 127)  # int8
    acc = qx @ q_weight  (int8 x int8 -> int32 accumulation)
    out = (acc + bias_i32) * deq_scale                                   # f32

Implementation notes:
  * Sharding: rows of x (M) are split across the 8 cores.  Column-parallel
    (the hint) would replicate the 128 MB x load + the quantize work on
    every core; row-parallel loads x once total and keeps all per-core
    work 1/8th.  No collective needed either way.
  * The PE has no int8 mode, but every int8 value is exactly representable
    in bf16 and the PSUM accumulates in fp32, which is exact for integer
    partial sums below 2^24 (|acc| here is ~1e5-1e6).  So a bf16 matmul of
    the quantized operands reproduces the int32 GEMM bit-exactly.
  * Quantize is split across two engines: the Act engine computes
    t = Identity(x*s + 0) (the affine is a single-rounding FMA; bias 0
    keeps it equal to a plain f32 multiply), the DVE does the fused
    round-half-even (t + 1.5*2^23 - 1.5*2^23) in one two-ALU pass.
  * x is staged chunk-major ([KT/4, 128, 4*MP]) so each x DMA moves a
    contiguous 2 MiB (small transfers only reach ~270 GB/s; >=2 MiB gets
    ~330+).  The first chunk is split so kt0 lands fast and the PE can
    start.  All weight loads ride the scalar-engine HWDGE ring so x
    streams uncontended on the sync ring.
  * Warm-up: 4 n-tiles accumulate (all 8 PSUM banks) with the k-loop
    outer, joining as their weight tile arrives (missed k-tiles are
    replayed at the end of phase 1 -- fp32 accumulation of <2^24-magnitude
    integers is exact in any order).  Steady state runs kt-outer/mb-inner
    so consecutive matmuls share the stationary tile.
  * Output is computed transposed ([N, M] per core) so bias/deq are
    per-partition scalars -> single fused tensor_scalar epilogue
    (acc + bias) * deq, one rounding, matching the reference exactly.
  * Host-side work is layout-only: transpose/slice x, int8->bf16 cast and
    swizzle of the weight, un-transpose of the output.
"""

import numpy as np
import ml_dtypes

_NC = 8  # NeuronCores
_P = 128  # partitions
_FREE = 512  # matmul moving free dim / PSUM bank (fp32)
_CH = 4  # k-tiles per x DMA chunk
_MAGIC = 12582912.0  # 1.5 * 2**23, RNE round-to-int magic constant

_nc_cache = {}


def _build_bass(
    MP, KT, NT, act_scale, act_offset, need_clip, nt_warm, body_reps=1, loop_trips=1,
    internal_io=False, prewarm=0,
):
    """Emit the per-core Bass/Tile program.

    DRAM tensors (per core):
      xt     [KT/CH, 128, CH*MP] f32  x-slice transposed, chunk-major
      w      [NT, 128, KT, 128] bf16  weight swizzled per n-tile
      bias_s [128, NT] f32   bias striped: [p, nt] = bias[nt*128 + p]
      deq_s  [128, NT] f32   deq striped likewise
      out    [NT, 128, MP] f32  transposed output: [nt, p, m] = y[m, nt*128+p]
    """
    from contextlib import ExitStack

    import concourse.mybir as mybir
    import concourse.tile as tile
    from concourse import bacc

    f32 = mybir.dt.float32
    bf16 = mybir.dt.bfloat16
    Alu = mybir.AluOpType
    Identity = mybir.ActivationFunctionType.Identity
    MB = MP // _FREE
    NCH = KT // _CH

    nc = bacc.Bacc("TRN2", target_bir_lowering=False, debug=False)

    # internal_io: timing-only mode.  The big tensors become DRAM scratch so
    # the benchmark harness stages ~nothing over the axon tunnel; a tiny
    # dummy in/out pair keeps the PJRT wrapper well-formed.  The emitted
    # compute/DMA stream is identical to the graded build.
    kin = "Internal" if internal_io else "ExternalInput"
    kout = "Internal" if internal_io else "ExternalOutput"
    xt_d = nc.dram_tensor("xt", [NCH, _P, _CH * MP], f32, kind=kin).ap()
    w_d = nc.dram_tensor("w", [NT, _P, KT, _P], bf16, kind=kin).ap()
    bias_d = nc.dram_tensor("bias_s", [_P, NT], f32, kind=kin).ap()
    deq_d = nc.dram_tensor("deq_s", [_P, NT], f32, kind=kin).ap()
    out_d = nc.dram_tensor("out", [NT, _P, MP], f32, kind=kout).ap()
    if internal_io:
        din = nc.dram_tensor("dummy_in", [1, 4], f32, kind="ExternalInput").ap()
        dout = nc.dram_tensor("dummy_out", [1, 4], f32, kind="ExternalOutput").ap()

    with tile.TileContext(nc) as tc, ExitStack() as ctx:
        const_pool = ctx.enter_context(tc.tile_pool(name="const", bufs=1))
        qx_pool = ctx.enter_context(tc.tile_pool(name="qxp", bufs=1))
        x_pool = ctx.enter_context(tc.tile_pool(name="xp", bufs=3))
        t_pool = ctx.enter_context(tc.tile_pool(name="tp", bufs=3))
        w_pool = ctx.enter_context(tc.tile_pool(name="wp", bufs=6))
        o_pool = ctx.enter_context(tc.tile_pool(name="op", bufs=6))
        ps_pool = ctx.enter_context(tc.tile_pool(name="pp", bufs=8, space="PSUM"))

        bias_t = const_pool.tile([_P, NT], f32, name="bias_t")
        nc.sync.dma_start(bias_t[:], bias_d)
        deq_t = const_pool.tile([_P, NT], f32, name="deq_t")
        nc.sync.dma_start(deq_t[:], deq_d)
        if internal_io:
            dummy_t = const_pool.tile([1, 4], f32, name="dummy_t")
            nc.sync.dma_start(dummy_t[:], din)
            nc.sync.dma_start(dout, dummy_t[:])

        # quantized-transposed activations, resident: [128, KT, MP] bf16
        qx = qx_pool.tile([_P, KT, MP], bf16, name="qx")

        pools = dict(
            x=x_pool, t=t_pool, w=w_pool, o=o_pool, ps=ps_pool,
        )

        def emit_reps():
            for _rep in range(body_reps):
                _emit_body(
                    nc, mybir, KT, NT, MP, MB, NCH, nt_warm,
                    act_scale, act_offset, need_clip,
                    qx, bias_t, deq_t,
                    xt_d, w_d, out_d, pools, prewarm,
                )

        if loop_trips > 1:
            with tc.For_i(0, loop_trips, 1):
                emit_reps()
        else:
            emit_reps()

    nc.compile()
    return nc


def _emit_body(
    nc, mybir, KT, NT, MP, MB, NCH, nt_warm,
    act_scale, act_offset, need_clip,
    qx, bias_t, deq_t,
    xt_d, w_d, out_d, pools, prewarm=0,
):
    f32 = mybir.dt.float32
    bf16 = mybir.dt.bfloat16
    Alu = mybir.AluOpType
    Identity = mybir.ActivationFunctionType.Identity

    def load_w(nt, split=False):
        # scalar-ring HWDGE: weight traffic never queues behind x chunks
        wt = pools["w"].tile([_P, KT, _P], bf16, name="wt")
        if split and KT > 4:
            # first k-block lands fast so the PE can start early
            nc.scalar.dma_start(wt[:, 0:4, :], w_d[nt][:, 0:4, :])
            nc.scalar.dma_start(wt[:, 4:, :], w_d[nt][:, 4:, :])
        else:
            nc.scalar.dma_start(wt[:], w_d[nt])
        return wt

    def quantize(xt_c, j, kt):
        # Act engine: t = Identity(x*s + 0) -- single-rounding FMA with
        # bias 0 == plain f32 multiply.  DVE: fused +MAGIC,-MAGIC RNE.
        t1 = pools["t"].tile([_P, MP], f32, name="t1")
        t2 = pools["t"].tile([_P, MP], f32, name="t2") if need_clip else None
        for h in range(2):
            hs = slice(h * (MP // 2), (h + 1) * (MP // 2))
            src = slice(j * MP + h * (MP // 2), j * MP + (h + 1) * (MP // 2))
            nc.scalar.activation(t1[:, hs], xt_c[:, src], Identity, 0.0, act_scale)
            if act_offset != 0.0:
                # reference computes mul and add with separate roundings
                nc.vector.tensor_scalar_add(t1[:, hs], t1[:, hs], act_offset)
            if need_clip:
                nc.vector.tensor_scalar(
                    t2[:, hs], t1[:, hs], _MAGIC, _MAGIC, Alu.add, Alu.subtract
                )
                nc.vector.tensor_scalar(
                    qx[:, kt, hs], t2[:, hs], 127.0, -128.0, Alu.min, Alu.max
                )
            else:
                nc.vector.tensor_scalar(
                    qx[:, kt, hs], t1[:, hs], _MAGIC, _MAGIC, Alu.add, Alu.subtract
                )

    def epilogue(nt, mb, ps):
        ot = pools["o"].tile([_P, _FREE], f32, name="ot")
        # (acc + bias) * deq, per-partition scalars, single instruction
        nc.vector.tensor_scalar(
            ot[:], ps[:], bias_t[:, nt : nt + 1], deq_t[:, nt : nt + 1],
            Alu.add, Alu.mult,
        )
        nc.sync.dma_start(out_d[nt, :, mb * _FREE : (mb + 1) * _FREE], ot[:])

    joins = tuple(min(3 * i, KT - 1) for i in range(nt_warm))

    warm_ps = [
        [pools["ps"].tile([_P, _FREE], f32, name="ps") for _ in range(MB)]
        for _ in range(nt_warm)
    ]

    # PE prewarm: tiny fp32 dummy matmuls on the resident bias tile keep
    # the HAM clock gate open through the DMA-bound startup window, so the
    # real matmuls start at 2.4 GHz instead of ramping from 1.2.  Warm
    # group 0's start=True scrubs the scratch bank; zero extra DMAs.
    # Must be emitted before the weight-load triggers (the reverse order
    # measured ~11µs slower -- scheduler/semaphore assignment artifact).
    pw_cols = min(32, NT)
    for _ in range(prewarm):
        nc.tensor.matmul(
            warm_ps[0][0][0:pw_cols, 0:pw_cols],
            bias_t[:, 0:pw_cols],
            bias_t[:, 0:pw_cols],
            start=True, stop=True,
        )

    warm_w = [load_w(nt, split=(nt == 0)) for nt in range(nt_warm)]

    def warm_mm(nt, k, start, stop):
        for mb in range(MB):
            nc.tensor.matmul(
                warm_ps[nt][mb][:], warm_w[nt][:, k, :],
                qx[:, k, mb * _FREE : (mb + 1) * _FREE],
                start=start, stop=stop,
            )

    kt = 0
    for c in range(NCH):
        xt_c = pools["x"].tile([_P, _CH * MP], f32, name="xt_c")
        if c == 0:
            # split so kt0 lands fast and the PE can start early
            nc.sync.dma_start(xt_c[:, 0:MP], xt_d[0][:, 0:MP])
            nc.sync.dma_start(xt_c[:, MP:], xt_d[0][:, MP:])
        else:
            nc.sync.dma_start(xt_c[:], xt_d[c])
        for j in range(_CH):
            quantize(xt_c, j, kt)
            for nt in range(nt_warm):
                fk = joins[nt]
                if kt < fk:
                    continue
                if kt == fk and fk > 0:
                    # join: current kt starts the group, then the whole
                    # backlog is emitted as ready filler work for the PE
                    # (fp32 accumulation of <2^24-magnitude integers is
                    # exact in any order).  If the join IS the last k-tile,
                    # the final backlog matmul closes the group.
                    warm_mm(nt, kt, start=True, stop=False)
                    for k in range(fk):
                        warm_mm(
                            nt, k, start=False,
                            stop=(fk == KT - 1 and k == fk - 1),
                        )
                else:
                    warm_mm(
                        nt, kt,
                        start=(kt == 0 and fk == 0),
                        stop=(kt == KT - 1),
                    )
            kt += 1
    for nt in range(nt_warm):
        for mb in range(MB):
            epilogue(nt, mb, warm_ps[nt][mb])

    # Steady state: kt outer / mb inner so consecutive matmuls share the
    # stationary W tile.  One PSUM bank per (nt, mb) accumulation group.
    # The last group runs mb outer so its first epilogue and store overlap
    # the remaining matmuls (drain trim).
    for nt in range(nt_warm, NT):
        wt = load_w(nt)
        pss = [pools["ps"].tile([_P, _FREE], f32, name="ps") for _ in range(MB)]
        if nt == NT - 1:
            for mb in range(MB):
                for k in range(KT):
                    nc.tensor.matmul(
                        pss[mb][:], wt[:, k, :],
                        qx[:, k, mb * _FREE : (mb + 1) * _FREE],
                        start=(k == 0), stop=(k == KT - 1),
                    )
                epilogue(nt, mb, pss[mb])
        else:
            for k in range(KT):
                for mb in range(MB):
                    nc.tensor.matmul(
                        pss[mb][:], wt[:, k, :],
                        qx[:, k, mb * _FREE : (mb + 1) * _FREE],
                        start=(k == 0), stop=(k == KT - 1),
                    )
            for mb in range(MB):
                epilogue(nt, mb, pss[mb])


def _stage_x(x, c, MP, KT):
    """Per-core chunk-major x: [KT/CH, 128, CH*MP]."""
    xs = x[c * MP : (c + 1) * MP, :].T.reshape(KT, _P, MP)
    return np.ascontiguousarray(
        xs.reshape(KT // _CH, _CH, _P, MP).transpose(0, 2, 1, 3).reshape(
            KT // _CH, _P, _CH * MP
        )
    )


def run(inputs, trace=False):
    """Full-input entry: shard, run on 8 cores, gather.  Returns (out, results)."""
    from concourse import bass_utils

    x = np.ascontiguousarray(np.asarray(inputs["x"], dtype=np.float32))
    qw = np.asarray(inputs["q_weight"])
    act_scale = float(np.asarray(inputs["act_scale"]))
    act_offset = float(np.asarray(inputs["act_offset"]))
    deq = np.asarray(inputs["deq_scale"], dtype=np.float32)
    bias = np.asarray(inputs["bias_i32"])

    M, K = x.shape
    K2, N = qw.shape
    assert K == K2 and M % _NC == 0
    MP = M // _NC
    assert MP % _FREE == 0 and K % (_P * _CH) == 0 and N % _P == 0
    KT, NT = K // _P, N // _P
    nt_warm = min(4, NT)

    # clip is a no-op unless |x*s + o| can reach 127.5; check the actual data
    t_max = float(np.abs(x).max()) * abs(act_scale) + abs(act_offset)
    need_clip = t_max >= 127.0
    assert np.abs(bias).max() < 2**24  # int32 bias must be f32-exact

    key = (MP, KT, NT, act_scale, act_offset, need_clip, nt_warm)
    if key not in _nc_cache:
        _nc_cache[key] = _build_bass(*key)
    nc = _nc_cache[key]

    # ---- host staging (layout-only transforms) ----
    # weight: int8 -> bf16 (exact), swizzled to [nt, p, kt, n]
    w_bf = qw.astype(ml_dtypes.bfloat16)
    w_r = np.ascontiguousarray(
        w_bf.reshape(KT, _P, NT, _P).transpose(2, 1, 0, 3)
    )
    bias_r = np.ascontiguousarray(bias.astype(np.float32).reshape(NT, _P).T)
    deq_r = np.ascontiguousarray(deq.reshape(NT, _P).T)

    in_maps = []
    for c in range(_NC):
        in_maps.append(
            {
                "xt": _stage_x(x, c, MP, KT),
                "w": w_r,
                "bias_s": bias_r,
                "deq_s": deq_r,
            }
        )

    results = bass_utils.run_bass_kernel_spmd(
        nc, in_maps, core_ids=list(range(_NC)), trace=trace
    )

    out = np.empty((M, N), dtype=np.float32)
    for c in range(_NC):
        out[c * MP : (c + 1) * MP, :] = results.results[c]["out"].reshape(N, MP).T
    return out, results


def kernel(**inputs) -> np.ndarray:
    out, _ = run(inputs, trace=False)
    return out
